# revision 37
# baseline (speedup 1.0000x reference)
"""Trainium2 Bass kernel for nn_BlockTrainerBlend (8-core data parallel).

Math (per batch row):
  split x0/x1/x2 into C=20 chunks of S=80; per (modality m, chunk c):
  proj = x_chunk @ W[m,c]^T + b[m,c]  -> [R*S=400]
  m = proj0*proj1*proj2; z = sum over r -> [80]
  z' = signed-sqrt(z); z_norm = z'/max(||z'||, eps)
  chunk_logits[c] = z_norm[c] @ Wo_c^T + b_out; chunks_out = softmax
  final = softmax(z_flat @ W_out^T + b_out)

Sharding: batch (2048) split 8 ways -> 256 rows/core, two 128-row tiles.
Weights replicated. All matmul operands pre-transposed/cast to fp16 on host,
with a ones-row appended so biases ride inside the matmuls (K=81).
"""
import numpy as np

import concourse.bacc as bacc
import concourse.bass as bass
import concourse.tile as tile
from concourse import mybir
from concourse.bass_utils import run_bass_kernel_spmd
from concourse.masks import make_identity

B, MM, C, S, R, O = 2048, 1600, 20, 80, 5, 27
NCORES = 8
BL = B // NCORES          # 256 rows per core
NT = BL // 128            # 2 batch-tiles per core

F32 = mybir.dt.float32
F16 = mybir.dt.float16
AF = mybir.ActivationFunctionType
ALU = mybir.AluOpType

_prog = None  # cached compiled Bass program


def _emit(nc, tc, ctx):
    # partition-major with chunk-contiguous columns: each 2-chunk group is
    # one contiguous 2D DMA pattern (81 rows x contiguous bytes)
    xT = nc.dram_tensor("xT", [81, C, 3, BL], F16, kind="ExternalInput").ap()
    Wb = nc.dram_tensor("Wb", [81, C, 3, R * S], F16, kind="ExternalInput").ap()
    WoT = nc.dram_tensor("WoT", [81, C, O], F16, kind="ExternalInput").ap()
    fin = nc.dram_tensor("fin", [BL, O], F32, kind="ExternalOutput").ap()
    chk = nc.dram_tensor("chk", [BL, C * O], F32, kind="ExternalOutput").ap()

    consts = ctx.enter_context(tc.tile_pool(name="consts", bufs=1))
    sb = ctx.enter_context(tc.tile_pool(name="sb", bufs=2))
    pb = ctx.enter_context(tc.tile_pool(name="pb", bufs=1))
    pp0 = ctx.enter_context(tc.tile_pool(name="pp0", bufs=2, space="PSUM"))
    pp = ctx.enter_context(tc.tile_pool(name="pp", bufs=4, space="PSUM"))
    pt = ctx.enter_context(tc.tile_pool(name="pt", bufs=1, space="PSUM"))
    ph = ctx.enter_context(tc.tile_pool(name="ph", bufs=1, space="PSUM"))

    xT_s = consts.tile([81, C, 3, BL], F16)
    Wb_s = consts.tile([81, C, 3, R * S], F16)
    nc.sync.dma_start(out=xT_s[:, 0:2], in_=xT[:, 0:2])
    nc.sync.dma_start(out=Wb_s[:, 0:2], in_=Wb[:, 0:2])
    WoT_s = consts.tile([81, C, O], F16)
    nc.sync.dma_start(out=WoT_s, in_=WoT)
    eb19 = nc.dram_tensor("eb19", [1, O], F32, kind="ExternalInput").ap()
    eb19_s = consts.tile([128, O], F32)
    nc.sync.dma_start(
        out=eb19_s,
        in_=bass.AP(tensor=eb19.tensor, offset=eb19.offset,
                    ap=[[0, 128], [1, O]]),
    )
    ident = consts.tile([128, 128], F16)
    make_identity(nc, ident)
    tiny_b = consts.tile([128, 1], F32)
    nc.vector.memset(tiny_b, 1e-30)
    # two alternating lhsT staging tiles for the head matmuls; bias row 80 is
    # written once and never touched again (copies only write rows 0..79)
    zTss = [
        pb.tile([81, 128], F16, name=f"zTs{i}", tag=f"zTs{i}") for i in range(3)
    ]
    for zz in zTss:
        nc.gpsimd.memset(zz[64:81, :], 1.0)
    # chunked input DMAs: chunk-c compute starts as soon as its slices land
    for c2 in range(1, C // 2):
        csl = slice(2 * c2, 2 * c2 + 2)
        nc.sync.dma_start(out=xT_s[:, csl], in_=xT[:, csl])
        nc.sync.dma_start(out=Wb_s[:, csl], in_=Wb[:, csl])

    zbufs = [
        pb.tile([128, C * S], F32, name=f"zbuf{t}", tag=f"zbuf{t}")
        for t in range(NT)
    ]
    mbufs = [
        pb.tile([128, C, R * S], F16, name=f"mbuf{t}", tag=f"mbuf{t}")
        for t in range(NT)
    ]
    znbs = [
        pb.tile([128, C * S], F16, name=f"znb{t}", tag=f"znb{t}")
        for t in range(NT)
    ]
    sas = [
        pb.tile([128, C], F32, name=f"sa{t}", tag=f"sa{t}") for t in range(NT)
    ]
    gs = [
        pb.tile([128, C], F16, name=f"g{t}", tag=f"g{t}") for t in range(NT)
    ]
    expbs = [
        pb.tile([128, C, O], F32, name=f"expb{t}", tag=f"expb{t}")
        for t in range(NT)
    ]
    pbufs = [
        pb.tile([128, C, O], F32, name=f"pbuf{t}", tag=f"pbuf{t}")
        for t in range(NT)
    ]

    def zred_group(t, c5):
        """Rank reduce on GpSimd (4 strided adds over a 5-chunk group)."""
        zbuf, mbuf = zbufs[t], mbufs[t]
        csl = slice(5 * c5, 5 * c5 + 5)
        mbv = mbuf.rearrange("p c (s r) -> p c s r", r=R)
        zbv = zbuf.rearrange("p (c s) -> p c s", s=S)
        tr1 = sb.tile([128, 5, S], F16, tag="tr1")
        tr2 = sb.tile([128, 5, S], F16, tag="tr2")
        nc.gpsimd.tensor_add(tr1, mbv[:, csl, :, 0], mbv[:, csl, :, 1])
        nc.gpsimd.tensor_add(tr2, mbv[:, csl, :, 2], mbv[:, csl, :, 3])
        nc.gpsimd.tensor_add(tr1, tr1, tr2)
        nc.gpsimd.tensor_add(zbv[:, csl, :], tr1, mbv[:, csl, :, 4])

    fbufs = {}

    def saB(t, c5):
        """|z| sums + rsqrt factors for a 5-chunk group (DVE reduce + ACT)."""
        zbuf = zbufs[t]
        sa, g = sas[t], gs[t]
        csl = slice(5 * c5, 5 * c5 + 5)
        esl = slice(5 * c5 * S, (5 * c5 + 5) * S)
        zbv = zbuf.rearrange("p (c s) -> p c s", s=S)
        fbuf = sb.tile([128, 5 * S], F32, tag="fbuf")
        fbufs[(t, c5)] = fbuf
        nc.scalar.activation(
            out=fbuf, in_=zbuf[:, esl], func=AF.Abs_reciprocal_sqrt,
            bias=tiny_b,
        )
        nc.vector.tensor_reduce(
            out=sa[:, csl], in_=zbv[:, csl], axis=mybir.AxisListType.X,
            op=ALU.add, apply_absolute_value=True,
        )
        # g = rsqrt(sum|z|) via the same Abs_reciprocal_sqrt table set
        nc.scalar.activation(
            out=g[:, csl], in_=sa[:, csl], func=AF.Abs_reciprocal_sqrt,
            bias=tiny_b,
        )

    def dveB(t, c5):
        """z_norm = (z * f) * g for a 5-chunk group (two batched DVE muls)."""
        zbuf, znb, g = zbufs[t], znbs[t], gs[t]
        csl = slice(5 * c5, 5 * c5 + 5)
        esl = slice(5 * c5 * S, (5 * c5 + 5) * S)
        fbuf = fbufs.pop((t, c5))
        zf = sb.tile([128, 5 * S], F16, tag="zf")
        nc.gpsimd.tensor_mul(zf, zbuf[:, esl], fbuf)
        gsl = g[:, csl]
        gb = bass.AP(
            tensor=gsl.tensor, offset=gsl.offset,
            ap=[gsl.ap[0], [1, 5], [0, S]],
        )
        nc.vector.tensor_mul(
            znb.rearrange("p (c s) -> p c s", s=S)[:, csl],
            zf.rearrange("p (c s) -> p c s", s=S), gb,
        )

    def head_group(t, c5):
        znb, expb = znbs[t], expbs[t]
        csl = slice(5 * c5, 5 * c5 + 5)
        # heads: 5 chunks share one PSUM bank -> one Exp per group
        P5_ps = ph.tile([128, 5, O], F32, tag="P")
        for j in range(5):
            c = 5 * c5 + j
            zT_ps = pt.tile([S, 128], F16, tag="zT")
            nc.tensor.transpose(zT_ps, znb[:, c * S:(c + 1) * S], ident)
            zTs = zTss[j % 3]
            nc.scalar.copy(zTs[:S, :], zT_ps)
            nc.tensor.matmul(
                P5_ps[:, j, :], lhsT=zTs, rhs=WoT_s[:, c, :],
                start=True, stop=True,
            )
        # evacuate logits; Exp happens once per tile in the epilogue so the
        # ACT table set doesn't thrash between ars and exp per group
        nc.scalar.copy(pbufs[t][:, csl, :], P5_ps)

    saq, dveq, headq = [], [], []
    # ---- main loop: projections + 3-way product, chunk-major across tiles,
    # with per-5-chunk-group tails pipelined in ----
    for c in range(C):
        for t in range(NT):
            bsl = slice(t * 128, (t + 1) * 128)
            mbuf = mbufs[t]
            proj = []
            for m in range(3):
                pool_m = pp0 if m == 0 else pp
                p = pool_m.tile(
                    [128, R * S], F32, tag="proj0" if m == 0 else "proj"
                )
                nc.tensor.matmul(
                    p, lhsT=xT_s[:, c, m, bsl], rhs=Wb_s[:, c, m, :],
                    start=True, stop=True,
                )
                proj.append(p)
            # DVE has a single PSUM read port: at most one PSUM operand per
            # tensor_tensor. Evacuate proj0 PSUM->SBUF on ScalarE first.
            p0c = sb.tile([128, R * S], F16, tag="p0c")
            nc.scalar.copy(p0c, proj[0])
            m01 = sb.tile([128, R * S], F32, tag="m01")
            nc.vector.tensor_mul(m01, p0c, proj[1])
            nc.vector.tensor_mul(mbuf[:, c, :], m01, proj[2])
            if c % 5 == 4:
                # advance the group pipeline one stage per boundary:
                # zred(now) -> saB(+1 boundary) -> dveB(+2) -> heads(+3)
                if headq:
                    head_group(*headq.pop(0))
                if dveq:
                    it = dveq.pop(0)
                    dveB(*it)
                    headq.append(it)
                if len(saq) >= 2:
                    it = saq.pop(0)
                    saB(*it)
                    dveq.append(it)
                zred_group(t, c // 5)
                saq.append((t, c // 5))

    # ---- drained tail: flush the group pipeline ----
    while saq or dveq or headq:
        if headq:
            head_group(*headq.pop(0))
        if dveq:
            it = dveq.pop(0)
            dveB(*it)
            headq.append(it)
        if saq:
            it = saq.pop(0)
            saB(*it)
            dveq.append(it)

    # ---- per-tile epilogue: softmaxes + stores ----
    for t in range(NT):
        bsl = slice(t * 128, (t + 1) * 128)
        expb = expbs[t]
        nc.scalar.activation(out=expb, in_=pbufs[t], func=AF.Exp)
        den = sb.tile([128, C], F32, tag="den")
        nc.vector.tensor_reduce(
            out=den, in_=expb, axis=mybir.AxisListType.X, op=ALU.add,
        )
        rden = sb.tile([128, C], F32, tag="rden")
        nc.vector.reciprocal(rden, den)
        outc = sb.tile([128, C * O], F32, tag="outc")
        rdb = bass.AP(
            tensor=rden.tensor, offset=rden.offset,
            ap=[rden.ap[0], [1, C], [0, O]],
        )
        nc.vector.tensor_mul(
            outc.rearrange("p (c o) -> p c o", o=O), expb, rdb,
        )
        nc.sync.dma_start(out=chk[bsl, :], in_=outc)
        # final logits: sum_c P_c = ln(prod_c e^{P_c}); the per-chunk bias
        # rows overcount b_out 20x -> correct with e^{-19*b_out}
        fprod = sb.tile([128, O], F32, tag="fprod")
        ebx = expb.rearrange("p c o -> p o c")
        nc.vector.tensor_reduce(
            out=fprod, in_=ebx, axis=mybir.AxisListType.X, op=ALU.mult,
        )
        fexp = sb.tile([128, O], F32, tag="fexp")
        nc.vector.tensor_mul(fexp, fprod, eb19_s)
        fden = sb.tile([128, 1], F32, tag="fden")
        nc.vector.tensor_reduce(
            out=fden, in_=fexp, axis=mybir.AxisListType.X, op=ALU.add,
        )
        rfden = sb.tile([128, 1], F32, tag="rfden")
        nc.vector.reciprocal(rfden, fden)
        outf = sb.tile([128, O], F32, tag="outf")
        nc.vector.tensor_scalar_mul(outf, fexp, rfden)
        nc.sync.dma_start(out=fin[bsl, :], in_=outf)


def build():
    global _prog
    if _prog is not None:
        return _prog
    nc = bacc.Bacc("TRN2", target_bir_lowering=False, debug=False)
    from contextlib import ExitStack

    with tile.TileContext(nc) as tc, ExitStack() as ctx:
        _emit(nc, tc, ctx)
    nc.compile()
    _prog = nc
    return nc


def _prep_inputs(x0, x1, x2, W, b, W_out, b_out):
    """Host-side shard + layout prep. Returns per-core input dicts."""
    xs = np.stack([x0, x1, x2]).astype(np.float32)       # [3, B, MM]
    src = xs.reshape(3, NCORES, BL, C, S)
    xTc = np.empty((NCORES, 81, C, 3, BL), np.float16)
    xTc[:, :S] = src.transpose(1, 4, 3, 0, 2)            # [core][i][c][m][u]
    xTc[:, S:] = 1.0

    W5 = W.reshape(3, C, R, S, S)
    Wb_a = np.empty((81, C, 3, R * S), np.float16)
    Wb_a[:S] = W5.transpose(4, 1, 0, 3, 2).reshape(S, C, 3, R * S)
    Wb_a[S] = (
        b.reshape(3, C, R, S).transpose(1, 0, 3, 2).reshape(C, 3, R * S)
    )

    WoT_a = np.empty((81, C, O), np.float16)
    WoT_a[:S] = W_out.reshape(O, C, S).transpose(2, 1, 0)
    WoT_a[S] = b_out[None, :]

    eb19 = np.exp(-19.0 * b_out.astype(np.float64)).astype(np.float32)[None, :]

    return [
        {"xT": xTc[i], "Wb": Wb_a, "WoT": WoT_a, "eb19": eb19}
        for i in range(NCORES)
    ]


def run(x0, x1, x2, W, b, W_out, b_out, trace=False):
    nc = build()
    in_maps = _prep_inputs(
        np.asarray(x0), np.asarray(x1), np.asarray(x2), np.asarray(W),
        np.asarray(b), np.asarray(W_out), np.asarray(b_out),
    )
    res = run_bass_kernel_spmd(nc, in_maps, core_ids=list(range(NCORES)), trace=trace)
    final = np.concatenate([r["fin"] for r in res.results], axis=0)
    chunks = np.concatenate(
        [r["chk"].reshape(BL, C, O) for r in res.results], axis=0
    )
    return (final, chunks), res


def kernel(x0, x1, x2, W, b, W_out, b_out):
    (final, chunks), _ = run(x0, x1, x2, W, b, W_out, b_out, trace=False)
    return final, chunks


# revision 38
# speedup vs baseline: 1.0384x; 1.0384x over previous
"""Trainium2 Bass kernel for nn_BlockTrainerBlend (8-core data parallel).

Math (per batch row):
  split x0/x1/x2 into C=20 chunks of S=80; per (modality m, chunk c):
  proj = x_chunk @ W[m,c]^T + b[m,c]  -> [R*S=400]
  m = proj0*proj1*proj2; z = sum over r -> [80]
  z' = signed-sqrt(z); z_norm = z'/max(||z'||, eps)
  chunk_logits[c] = z_norm[c] @ Wo_c^T + b_out; chunks_out = softmax
  final = softmax(z_flat @ W_out^T + b_out)

Sharding: batch (2048) split 8 ways -> 256 rows/core, two 128-row tiles.
Weights replicated. All matmul operands pre-transposed/cast to fp16 on host,
with a ones-row appended so biases ride inside the matmuls (K=81).
"""
import numpy as np

import concourse.bacc as bacc
import concourse.bass as bass
import concourse.tile as tile
from concourse import mybir
from concourse.bass_utils import run_bass_kernel_spmd
from concourse.masks import make_identity

B, MM, C, S, R, O = 2048, 1600, 20, 80, 5, 27
NCORES = 8
BL = B // NCORES          # 256 rows per core
NT = BL // 128            # 2 batch-tiles per core

F32 = mybir.dt.float32
F16 = mybir.dt.float16
AF = mybir.ActivationFunctionType
ALU = mybir.AluOpType

_prog = None  # cached compiled Bass program


def _emit(nc, tc, ctx):
    # partition-major with chunk-contiguous columns: each 2-chunk group is
    # one contiguous 2D DMA pattern (81 rows x contiguous bytes)
    xT = nc.dram_tensor("xT", [81, C, 3, BL], F16, kind="ExternalInput").ap()
    Wb = nc.dram_tensor("Wb", [81, C, 3, R * S], F16, kind="ExternalInput").ap()
    WoT = nc.dram_tensor("WoT", [81, C, O], F16, kind="ExternalInput").ap()
    fin = nc.dram_tensor("fin", [BL, O], F32, kind="ExternalOutput").ap()
    chk = nc.dram_tensor("chk", [BL, C * O], F32, kind="ExternalOutput").ap()

    consts = ctx.enter_context(tc.tile_pool(name="consts", bufs=1))
    sb = ctx.enter_context(tc.tile_pool(name="sb", bufs=2))
    pb = ctx.enter_context(tc.tile_pool(name="pb", bufs=1))
    pp0 = ctx.enter_context(tc.tile_pool(name="pp0", bufs=2, space="PSUM"))
    pp = ctx.enter_context(tc.tile_pool(name="pp", bufs=4, space="PSUM"))
    pt = ctx.enter_context(tc.tile_pool(name="pt", bufs=1, space="PSUM"))
    ph = ctx.enter_context(tc.tile_pool(name="ph", bufs=1, space="PSUM"))

    xT_s = consts.tile([81, C, 3, BL], F16)
    Wb_s = consts.tile([81, C, 3, R * S], F16)
    nc.sync.dma_start(out=xT_s[:, 0:2], in_=xT[:, 0:2])
    nc.sync.dma_start(out=Wb_s[:, 0:2], in_=Wb[:, 0:2])
    WoT_s = consts.tile([81, C, O], F16)
    nc.sync.dma_start(out=WoT_s, in_=WoT)
    eb19 = nc.dram_tensor("eb19", [1, O], F32, kind="ExternalInput").ap()
    eb19_s = consts.tile([128, O], F32)
    nc.sync.dma_start(
        out=eb19_s,
        in_=bass.AP(tensor=eb19.tensor, offset=eb19.offset,
                    ap=[[0, 128], [1, O]]),
    )
    ident = consts.tile([128, 128], F16)
    make_identity(nc, ident)
    tiny_b = consts.tile([128, 1], F32)
    nc.vector.memset(tiny_b, 1e-30)
    # two alternating lhsT staging tiles for the head matmuls; bias row 80 is
    # written once and never touched again (copies only write rows 0..79)
    zTss = [
        pb.tile([81, 128], F16, name=f"zTs{i}", tag=f"zTs{i}") for i in range(3)
    ]
    for zz in zTss:
        nc.gpsimd.memset(zz[64:81, :], 1.0)
    # chunked input DMAs: chunk-c compute starts as soon as its slices land
    for c2 in range(1, C // 2):
        csl = slice(2 * c2, 2 * c2 + 2)
        nc.sync.dma_start(out=xT_s[:, csl], in_=xT[:, csl])
        nc.sync.dma_start(out=Wb_s[:, csl], in_=Wb[:, csl])

    zbufs = [
        pb.tile([128, C * S], F32, name=f"zbuf{t}", tag=f"zbuf{t}")
        for t in range(NT)
    ]
    mbufs = [
        pb.tile([128, C, R * S], F16, name=f"mbuf{t}", tag=f"mbuf{t}")
        for t in range(NT)
    ]
    znbs = [
        pb.tile([128, C * S], F16, name=f"znb{t}", tag=f"znb{t}")
        for t in range(NT)
    ]
    sas = [
        pb.tile([128, C], F32, name=f"sa{t}", tag=f"sa{t}") for t in range(NT)
    ]
    gs = [
        pb.tile([128, C], F16, name=f"g{t}", tag=f"g{t}") for t in range(NT)
    ]
    expbs = [
        pb.tile([128, C, O], F32, name=f"expb{t}", tag=f"expb{t}")
        for t in range(NT)
    ]
    pbufs = [
        pb.tile([128, C, O], F32, name=f"pbuf{t}", tag=f"pbuf{t}")
        for t in range(NT)
    ]

    def zred_group(t, c5):
        """Rank reduce on GpSimd (4 strided adds over a 5-chunk group)."""
        zbuf, mbuf = zbufs[t], mbufs[t]
        csl = slice(5 * c5, 5 * c5 + 5)
        mbv = mbuf.rearrange("p c (s r) -> p c s r", r=R)
        zbv = zbuf.rearrange("p (c s) -> p c s", s=S)
        tr1 = sb.tile([128, 5, S], F16, tag="tr1")
        tr2 = sb.tile([128, 5, S], F16, tag="tr2")
        nc.gpsimd.tensor_add(tr1, mbv[:, csl, :, 0], mbv[:, csl, :, 1])
        nc.gpsimd.tensor_add(tr2, mbv[:, csl, :, 2], mbv[:, csl, :, 3])
        nc.gpsimd.tensor_add(tr1, tr1, tr2)
        nc.gpsimd.tensor_add(zbv[:, csl, :], tr1, mbv[:, csl, :, 4])

    fbufs = {}

    def saB(t, c5):
        """|z| sums + rsqrt factors for a 5-chunk group (DVE reduce + ACT)."""
        zbuf = zbufs[t]
        sa, g = sas[t], gs[t]
        csl = slice(5 * c5, 5 * c5 + 5)
        esl = slice(5 * c5 * S, (5 * c5 + 5) * S)
        zbv = zbuf.rearrange("p (c s) -> p c s", s=S)
        fbuf = sb.tile([128, 5 * S], F32, tag="fbuf")
        fbufs[(t, c5)] = fbuf
        nc.scalar.activation(
            out=fbuf, in_=zbuf[:, esl], func=AF.Abs_reciprocal_sqrt,
            bias=tiny_b,
        )
        nc.vector.tensor_reduce(
            out=sa[:, csl], in_=zbv[:, csl], axis=mybir.AxisListType.X,
            op=ALU.add, apply_absolute_value=True,
        )
        # g = rsqrt(sum|z|) via the same Abs_reciprocal_sqrt table set
        nc.scalar.activation(
            out=g[:, csl], in_=sa[:, csl], func=AF.Abs_reciprocal_sqrt,
            bias=tiny_b,
        )

    def dveB(t, c5):
        """z_norm = (z * f) * g for a 5-chunk group (two batched DVE muls)."""
        zbuf, znb, g = zbufs[t], znbs[t], gs[t]
        csl = slice(5 * c5, 5 * c5 + 5)
        esl = slice(5 * c5 * S, (5 * c5 + 5) * S)
        fbuf = fbufs.pop((t, c5))
        zf = sb.tile([128, 5 * S], F16, tag="zf")
        nc.vector.tensor_mul(zf, zbuf[:, esl], fbuf)
        gsl = g[:, csl]
        gb = bass.AP(
            tensor=gsl.tensor, offset=gsl.offset,
            ap=[gsl.ap[0], [1, 5], [0, S]],
        )
        nc.vector.tensor_mul(
            znb.rearrange("p (c s) -> p c s", s=S)[:, csl],
            zf.rearrange("p (c s) -> p c s", s=S), gb,
        )

    def head_group(t, c5):
        znb, expb = znbs[t], expbs[t]
        csl = slice(5 * c5, 5 * c5 + 5)
        # heads: 5 chunks share one PSUM bank -> one Exp per group
        P5_ps = ph.tile([128, 5, O], F32, tag="P")
        for j in range(5):
            c = 5 * c5 + j
            zT_ps = pt.tile([S, 128], F16, tag="zT")
            nc.tensor.transpose(zT_ps, znb[:, c * S:(c + 1) * S], ident)
            zTs = zTss[j % 3]
            if j % 3 != 2:
                nc.scalar.copy(zTs[:S, :], zT_ps)
            else:
                nc.vector.tensor_copy(zTs[:S, :], zT_ps)
            nc.tensor.matmul(
                P5_ps[:, j, :], lhsT=zTs, rhs=WoT_s[:, c, :],
                start=True, stop=True,
            )
        # evacuate logits; Exp happens once per tile in the epilogue so the
        # ACT table set doesn't thrash between ars and exp per group
        nc.scalar.copy(pbufs[t][:, csl, :], P5_ps)

    saq, dveq, headq = [], [], []
    # ---- main loop: projections + 3-way product, chunk-major across tiles,
    # with per-5-chunk-group tails pipelined in ----
    for c in range(C):
        for t in range(NT):
            bsl = slice(t * 128, (t + 1) * 128)
            mbuf = mbufs[t]
            proj = []
            for m in range(3):
                pool_m = pp0 if m == 0 else pp
                p = pool_m.tile(
                    [128, R * S], F32, tag="proj0" if m == 0 else "proj"
                )
                nc.tensor.matmul(
                    p, lhsT=xT_s[:, c, m, bsl], rhs=Wb_s[:, c, m, :],
                    start=True, stop=True,
                )
                proj.append(p)
            # DVE has a single PSUM read port: at most one PSUM operand per
            # tensor_tensor. Evacuate proj0 PSUM->SBUF on ScalarE first.
            p0c = sb.tile([128, R * S], F16, tag="p0c")
            nc.scalar.copy(p0c, proj[0])
            m01 = sb.tile([128, R * S], F32, tag="m01")
            nc.vector.tensor_mul(m01, p0c, proj[1])
            nc.vector.tensor_mul(mbuf[:, c, :], m01, proj[2])
            if c % 5 == 4:
                # advance the group pipeline one stage per boundary:
                # zred(now) -> saB(+1 boundary) -> dveB(+2) -> heads(+3)
                if headq:
                    head_group(*headq.pop(0))
                if dveq:
                    it = dveq.pop(0)
                    dveB(*it)
                    headq.append(it)
                if saq:
                    it = saq.pop(0)
                    saB(*it)
                    dveq.append(it)
                zred_group(t, c // 5)
                saq.append((t, c // 5))

    # ---- drained tail: flush the group pipeline ----
    while saq or dveq or headq:
        if headq:
            head_group(*headq.pop(0))
        if dveq:
            it = dveq.pop(0)
            dveB(*it)
            headq.append(it)
        if saq:
            it = saq.pop(0)
            saB(*it)
            dveq.append(it)

    # ---- per-tile epilogue: softmaxes + stores ----
    for t in range(NT):
        bsl = slice(t * 128, (t + 1) * 128)
        expb = expbs[t]
        nc.scalar.activation(out=expb, in_=pbufs[t], func=AF.Exp)
        den = sb.tile([128, C], F32, tag="den")
        nc.vector.tensor_reduce(
            out=den, in_=expb, axis=mybir.AxisListType.X, op=ALU.add,
        )
        rden = sb.tile([128, C], F32, tag="rden")
        nc.vector.reciprocal(rden, den)
        outc = sb.tile([128, C * O], F32, tag="outc")
        rdb = bass.AP(
            tensor=rden.tensor, offset=rden.offset,
            ap=[rden.ap[0], [1, C], [0, O]],
        )
        nc.vector.tensor_mul(
            outc.rearrange("p (c o) -> p c o", o=O), expb, rdb,
        )
        nc.sync.dma_start(out=chk[bsl, :], in_=outc)
        # final logits: sum_c P_c = ln(prod_c e^{P_c}); the per-chunk bias
        # rows overcount b_out 20x -> correct with e^{-19*b_out}
        fprod = sb.tile([128, O], F32, tag="fprod")
        ebx = expb.rearrange("p c o -> p o c")
        nc.vector.tensor_reduce(
            out=fprod, in_=ebx, axis=mybir.AxisListType.X, op=ALU.mult,
        )
        fexp = sb.tile([128, O], F32, tag="fexp")
        nc.vector.tensor_mul(fexp, fprod, eb19_s)
        fden = sb.tile([128, 1], F32, tag="fden")
        nc.vector.tensor_reduce(
            out=fden, in_=fexp, axis=mybir.AxisListType.X, op=ALU.add,
        )
        rfden = sb.tile([128, 1], F32, tag="rfden")
        nc.vector.reciprocal(rfden, fden)
        outf = sb.tile([128, O], F32, tag="outf")
        nc.vector.tensor_scalar_mul(outf, fexp, rfden)
        nc.sync.dma_start(out=fin[bsl, :], in_=outf)


def build():
    global _prog
    if _prog is not None:
        return _prog
    nc = bacc.Bacc("TRN2", target_bir_lowering=False, debug=False)
    from contextlib import ExitStack

    with tile.TileContext(nc) as tc, ExitStack() as ctx:
        _emit(nc, tc, ctx)
    nc.compile()
    _prog = nc
    return nc


def _prep_inputs(x0, x1, x2, W, b, W_out, b_out):
    """Host-side shard + layout prep. Returns per-core input dicts."""
    xs = np.stack([x0, x1, x2]).astype(np.float32)       # [3, B, MM]
    src = xs.reshape(3, NCORES, BL, C, S)
    xTc = np.empty((NCORES, 81, C, 3, BL), np.float16)
    xTc[:, :S] = src.transpose(1, 4, 3, 0, 2)            # [core][i][c][m][u]
    xTc[:, S:] = 1.0

    W5 = W.reshape(3, C, R, S, S)
    Wb_a = np.empty((81, C, 3, R * S), np.float16)
    Wb_a[:S] = W5.transpose(4, 1, 0, 3, 2).reshape(S, C, 3, R * S)
    Wb_a[S] = (
        b.reshape(3, C, R, S).transpose(1, 0, 3, 2).reshape(C, 3, R * S)
    )

    WoT_a = np.empty((81, C, O), np.float16)
    WoT_a[:S] = W_out.reshape(O, C, S).transpose(2, 1, 0)
    WoT_a[S] = b_out[None, :]

    eb19 = np.exp(-19.0 * b_out.astype(np.float64)).astype(np.float32)[None, :]

    return [
        {"xT": xTc[i], "Wb": Wb_a, "WoT": WoT_a, "eb19": eb19}
        for i in range(NCORES)
    ]


def run(x0, x1, x2, W, b, W_out, b_out, trace=False):
    nc = build()
    in_maps = _prep_inputs(
        np.asarray(x0), np.asarray(x1), np.asarray(x2), np.asarray(W),
        np.asarray(b), np.asarray(W_out), np.asarray(b_out),
    )
    res = run_bass_kernel_spmd(nc, in_maps, core_ids=list(range(NCORES)), trace=trace)
    final = np.concatenate([r["fin"] for r in res.results], axis=0)
    chunks = np.concatenate(
        [r["chk"].reshape(BL, C, O) for r in res.results], axis=0
    )
    return (final, chunks), res


def kernel(x0, x1, x2, W, b, W_out, b_out):
    (final, chunks), _ = run(x0, x1, x2, W, b, W_out, b_out, trace=False)
    return final, chunks


# revision 39
# speedup vs baseline: 1.0467x; 1.0079x over previous
"""Trainium2 Bass kernel for nn_BlockTrainerBlend (8-core data parallel).

Math (per batch row):
  split x0/x1/x2 into C=20 chunks of S=80; per (modality m, chunk c):
  proj = x_chunk @ W[m,c]^T + b[m,c]  -> [R*S=400]
  m = proj0*proj1*proj2; z = sum over r -> [80]
  z' = signed-sqrt(z); z_norm = z'/max(||z'||, eps)
  chunk_logits[c] = z_norm[c] @ Wo_c^T + b_out; chunks_out = softmax
  final = softmax(z_flat @ W_out^T + b_out)

Sharding: batch (2048) split 8 ways -> 256 rows/core, two 128-row tiles.
Weights replicated. All matmul operands pre-transposed/cast to fp16 on host,
with a ones-row appended so biases ride inside the matmuls (K=81).
"""
import numpy as np

import concourse.bacc as bacc
import concourse.bass as bass
import concourse.tile as tile
from concourse import mybir
from concourse.bass_utils import run_bass_kernel_spmd
from concourse.masks import make_identity

B, MM, C, S, R, O = 2048, 1600, 20, 80, 5, 27
NCORES = 8
BL = B // NCORES          # 256 rows per core
NT = BL // 128            # 2 batch-tiles per core

F32 = mybir.dt.float32
F16 = mybir.dt.float16
AF = mybir.ActivationFunctionType
ALU = mybir.AluOpType

_prog = None  # cached compiled Bass program


def _emit(nc, tc, ctx):
    # partition-major with chunk-contiguous columns: each 2-chunk group is
    # one contiguous 2D DMA pattern (81 rows x contiguous bytes)
    xT = nc.dram_tensor("xT", [81, C, 3, BL], F16, kind="ExternalInput").ap()
    Wb = nc.dram_tensor("Wb", [81, C, 3, R * S], F16, kind="ExternalInput").ap()
    WoT = nc.dram_tensor("WoT", [81, C, O], F16, kind="ExternalInput").ap()
    fin = nc.dram_tensor("fin", [BL, O], F32, kind="ExternalOutput").ap()
    chk = nc.dram_tensor("chk", [BL, C * O], F32, kind="ExternalOutput").ap()

    consts = ctx.enter_context(tc.tile_pool(name="consts", bufs=1))
    sb = ctx.enter_context(tc.tile_pool(name="sb", bufs=2))
    pb = ctx.enter_context(tc.tile_pool(name="pb", bufs=1))
    pp0 = ctx.enter_context(tc.tile_pool(name="pp0", bufs=2, space="PSUM"))
    pp = ctx.enter_context(tc.tile_pool(name="pp", bufs=3, space="PSUM"))
    pt = ctx.enter_context(tc.tile_pool(name="pt", bufs=2, space="PSUM"))
    ph = ctx.enter_context(tc.tile_pool(name="ph", bufs=1, space="PSUM"))

    xT_s = consts.tile([81, C, 3, BL], F16)
    Wb_s = consts.tile([81, C, 3, R * S], F16)
    nc.sync.dma_start(out=xT_s[:, 0:2], in_=xT[:, 0:2])
    nc.sync.dma_start(out=Wb_s[:, 0:2], in_=Wb[:, 0:2])
    WoT_s = consts.tile([81, C, O], F16)
    nc.sync.dma_start(out=WoT_s, in_=WoT)
    eb19 = nc.dram_tensor("eb19", [1, O], F32, kind="ExternalInput").ap()
    eb19_s = consts.tile([128, O], F32)
    nc.sync.dma_start(
        out=eb19_s,
        in_=bass.AP(tensor=eb19.tensor, offset=eb19.offset,
                    ap=[[0, 128], [1, O]]),
    )
    ident = consts.tile([128, 128], F16)
    make_identity(nc, ident)
    tiny_b = consts.tile([128, 1], F32)
    nc.vector.memset(tiny_b, 1e-30)
    # two alternating lhsT staging tiles for the head matmuls; bias row 80 is
    # written once and never touched again (copies only write rows 0..79)
    zTss = [
        pb.tile([81, 128], F16, name=f"zTs{i}", tag=f"zTs{i}") for i in range(3)
    ]
    for zz in zTss:
        nc.gpsimd.memset(zz[64:81, :], 1.0)
    # chunked input DMAs: chunk-c compute starts as soon as its slices land
    for c2 in range(1, C // 2):
        csl = slice(2 * c2, 2 * c2 + 2)
        nc.sync.dma_start(out=xT_s[:, csl], in_=xT[:, csl])
        nc.sync.dma_start(out=Wb_s[:, csl], in_=Wb[:, csl])

    zbufs = [
        pb.tile([128, C * S], F32, name=f"zbuf{t}", tag=f"zbuf{t}")
        for t in range(NT)
    ]
    mbufs = [
        pb.tile([128, C, R * S], F16, name=f"mbuf{t}", tag=f"mbuf{t}")
        for t in range(NT)
    ]
    znbs = [
        pb.tile([128, C * S], F16, name=f"znb{t}", tag=f"znb{t}")
        for t in range(NT)
    ]
    sas = [
        pb.tile([128, C], F32, name=f"sa{t}", tag=f"sa{t}") for t in range(NT)
    ]
    gs = [
        pb.tile([128, C], F16, name=f"g{t}", tag=f"g{t}") for t in range(NT)
    ]
    expbs = [
        pb.tile([128, C, O], F32, name=f"expb{t}", tag=f"expb{t}")
        for t in range(NT)
    ]
    pbufs = [
        pb.tile([128, C, O], F32, name=f"pbuf{t}", tag=f"pbuf{t}")
        for t in range(NT)
    ]

    def zred_group(t, c5):
        """Rank reduce on GpSimd (4 strided adds over a 5-chunk group)."""
        zbuf, mbuf = zbufs[t], mbufs[t]
        csl = slice(5 * c5, 5 * c5 + 5)
        mbv = mbuf.rearrange("p c (s r) -> p c s r", r=R)
        zbv = zbuf.rearrange("p (c s) -> p c s", s=S)
        tr1 = sb.tile([128, 5, S], F16, tag="tr1")
        tr2 = sb.tile([128, 5, S], F16, tag="tr2")
        nc.gpsimd.tensor_add(tr1, mbv[:, csl, :, 0], mbv[:, csl, :, 1])
        nc.gpsimd.tensor_add(tr2, mbv[:, csl, :, 2], mbv[:, csl, :, 3])
        nc.gpsimd.tensor_add(tr1, tr1, tr2)
        nc.gpsimd.tensor_add(zbv[:, csl, :], tr1, mbv[:, csl, :, 4])

    fbufs = {}

    def saB(t, c5):
        """|z| sums + rsqrt factors for a 5-chunk group (DVE reduce + ACT)."""
        zbuf = zbufs[t]
        sa, g = sas[t], gs[t]
        csl = slice(5 * c5, 5 * c5 + 5)
        esl = slice(5 * c5 * S, (5 * c5 + 5) * S)
        zbv = zbuf.rearrange("p (c s) -> p c s", s=S)
        fbuf = sb.tile([128, 5 * S], F32, tag="fbuf")
        fbufs[(t, c5)] = fbuf
        nc.scalar.activation(
            out=fbuf, in_=zbuf[:, esl], func=AF.Abs_reciprocal_sqrt,
            bias=tiny_b,
        )
        nc.vector.tensor_reduce(
            out=sa[:, csl], in_=zbv[:, csl], axis=mybir.AxisListType.X,
            op=ALU.add, apply_absolute_value=True,
        )
        # g = rsqrt(sum|z|) via the same Abs_reciprocal_sqrt table set
        nc.scalar.activation(
            out=g[:, csl], in_=sa[:, csl], func=AF.Abs_reciprocal_sqrt,
            bias=tiny_b,
        )

    def dveB(t, c5):
        """z_norm = (z * f) * g for a 5-chunk group (two batched DVE muls)."""
        zbuf, znb, g = zbufs[t], znbs[t], gs[t]
        csl = slice(5 * c5, 5 * c5 + 5)
        esl = slice(5 * c5 * S, (5 * c5 + 5) * S)
        fbuf = fbufs.pop((t, c5))
        zf = sb.tile([128, 5 * S], F16, tag="zf")
        nc.vector.tensor_mul(zf, zbuf[:, esl], fbuf)
        gsl = g[:, csl]
        gb = bass.AP(
            tensor=gsl.tensor, offset=gsl.offset,
            ap=[gsl.ap[0], [1, 5], [0, S]],
        )
        nc.vector.tensor_mul(
            znb.rearrange("p (c s) -> p c s", s=S)[:, csl],
            zf.rearrange("p (c s) -> p c s", s=S), gb,
        )

    def head_group(t, c5):
        znb, expb = znbs[t], expbs[t]
        csl = slice(5 * c5, 5 * c5 + 5)
        # heads: 5 chunks share one PSUM bank -> one Exp per group
        P5_ps = ph.tile([128, 5, O], F32, tag="P")
        for j in range(5):
            c = 5 * c5 + j
            zT_ps = pt.tile([S, 128], F16, tag="zT")
            nc.tensor.transpose(zT_ps, znb[:, c * S:(c + 1) * S], ident)
            zTs = zTss[j % 3]
            if j % 3 != 2:
                nc.scalar.copy(zTs[:S, :], zT_ps)
            else:
                nc.vector.tensor_copy(zTs[:S, :], zT_ps)
            nc.tensor.matmul(
                P5_ps[:, j, :], lhsT=zTs, rhs=WoT_s[:, c, :],
                start=True, stop=True,
            )
        # evacuate logits; Exp happens once per tile in the epilogue so the
        # ACT table set doesn't thrash between ars and exp per group
        nc.scalar.copy(pbufs[t][:, csl, :], P5_ps)

    saq, dveq, headq = [], [], []
    # ---- main loop: projections + 3-way product, chunk-major across tiles,
    # with per-5-chunk-group tails pipelined in ----
    for c in range(C):
        for t in range(NT):
            bsl = slice(t * 128, (t + 1) * 128)
            mbuf = mbufs[t]
            proj = []
            for m in range(3):
                pool_m = pp0 if m == 0 else pp
                p = pool_m.tile(
                    [128, R * S], F32, tag="proj0" if m == 0 else "proj"
                )
                nc.tensor.matmul(
                    p, lhsT=xT_s[:, c, m, bsl], rhs=Wb_s[:, c, m, :],
                    start=True, stop=True,
                )
                proj.append(p)
            # DVE has a single PSUM read port: at most one PSUM operand per
            # tensor_tensor. Evacuate proj0 PSUM->SBUF on ScalarE first.
            p0c = sb.tile([128, R * S], F16, tag="p0c")
            nc.scalar.copy(p0c, proj[0])
            m01 = sb.tile([128, R * S], F32, tag="m01")
            nc.vector.tensor_mul(m01, p0c, proj[1])
            nc.vector.tensor_mul(mbuf[:, c, :], m01, proj[2])
            if c % 5 == 4:
                # advance the group pipeline one stage per boundary:
                # zred(now) -> saB(+1 boundary) -> dveB(+2) -> heads(+3)
                if headq:
                    head_group(*headq.pop(0))
                if dveq:
                    it = dveq.pop(0)
                    dveB(*it)
                    headq.append(it)
                if saq:
                    it = saq.pop(0)
                    saB(*it)
                    dveq.append(it)
                zred_group(t, c // 5)
                saq.append((t, c // 5))

    # ---- drained tail: flush the group pipeline ----
    while saq or dveq or headq:
        if headq:
            head_group(*headq.pop(0))
        if dveq:
            it = dveq.pop(0)
            dveB(*it)
            headq.append(it)
        if saq:
            it = saq.pop(0)
            saB(*it)
            dveq.append(it)

    # ---- per-tile epilogue: softmaxes + stores ----
    for t in range(NT):
        bsl = slice(t * 128, (t + 1) * 128)
        expb = expbs[t]
        nc.scalar.activation(out=expb, in_=pbufs[t], func=AF.Exp)
        den = sb.tile([128, C], F32, tag="den")
        nc.vector.tensor_reduce(
            out=den, in_=expb, axis=mybir.AxisListType.X, op=ALU.add,
        )
        rden = sb.tile([128, C], F32, tag="rden")
        nc.vector.reciprocal(rden, den)
        outc = sb.tile([128, C * O], F32, tag="outc")
        rdb = bass.AP(
            tensor=rden.tensor, offset=rden.offset,
            ap=[rden.ap[0], [1, C], [0, O]],
        )
        nc.vector.tensor_mul(
            outc.rearrange("p (c o) -> p c o", o=O), expb, rdb,
        )
        nc.sync.dma_start(out=chk[bsl, :], in_=outc)
        # final logits: sum_c P_c = ln(prod_c e^{P_c}); the per-chunk bias
        # rows overcount b_out 20x -> correct with e^{-19*b_out}
        fprod = sb.tile([128, O], F32, tag="fprod")
        ebx = expb.rearrange("p c o -> p o c")
        nc.vector.tensor_reduce(
            out=fprod, in_=ebx, axis=mybir.AxisListType.X, op=ALU.mult,
        )
        fexp = sb.tile([128, O], F32, tag="fexp")
        nc.vector.tensor_mul(fexp, fprod, eb19_s)
        fden = sb.tile([128, 1], F32, tag="fden")
        nc.vector.tensor_reduce(
            out=fden, in_=fexp, axis=mybir.AxisListType.X, op=ALU.add,
        )
        rfden = sb.tile([128, 1], F32, tag="rfden")
        nc.vector.reciprocal(rfden, fden)
        outf = sb.tile([128, O], F32, tag="outf")
        nc.vector.tensor_scalar_mul(outf, fexp, rfden)
        nc.sync.dma_start(out=fin[bsl, :], in_=outf)


def build():
    global _prog
    if _prog is not None:
        return _prog
    nc = bacc.Bacc("TRN2", target_bir_lowering=False, debug=False)
    from contextlib import ExitStack

    with tile.TileContext(nc) as tc, ExitStack() as ctx:
        _emit(nc, tc, ctx)
    nc.compile()
    _prog = nc
    return nc


def _prep_inputs(x0, x1, x2, W, b, W_out, b_out):
    """Host-side shard + layout prep. Returns per-core input dicts."""
    xs = np.stack([x0, x1, x2]).astype(np.float32)       # [3, B, MM]
    src = xs.reshape(3, NCORES, BL, C, S)
    xTc = np.empty((NCORES, 81, C, 3, BL), np.float16)
    xTc[:, :S] = src.transpose(1, 4, 3, 0, 2)            # [core][i][c][m][u]
    xTc[:, S:] = 1.0

    W5 = W.reshape(3, C, R, S, S)
    Wb_a = np.empty((81, C, 3, R * S), np.float16)
    Wb_a[:S] = W5.transpose(4, 1, 0, 3, 2).reshape(S, C, 3, R * S)
    Wb_a[S] = (
        b.reshape(3, C, R, S).transpose(1, 0, 3, 2).reshape(C, 3, R * S)
    )

    WoT_a = np.empty((81, C, O), np.float16)
    WoT_a[:S] = W_out.reshape(O, C, S).transpose(2, 1, 0)
    WoT_a[S] = b_out[None, :]

    eb19 = np.exp(-19.0 * b_out.astype(np.float64)).astype(np.float32)[None, :]

    return [
        {"xT": xTc[i], "Wb": Wb_a, "WoT": WoT_a, "eb19": eb19}
        for i in range(NCORES)
    ]


def run(x0, x1, x2, W, b, W_out, b_out, trace=False):
    nc = build()
    in_maps = _prep_inputs(
        np.asarray(x0), np.asarray(x1), np.asarray(x2), np.asarray(W),
        np.asarray(b), np.asarray(W_out), np.asarray(b_out),
    )
    res = run_bass_kernel_spmd(nc, in_maps, core_ids=list(range(NCORES)), trace=trace)
    final = np.concatenate([r["fin"] for r in res.results], axis=0)
    chunks = np.concatenate(
        [r["chk"].reshape(BL, C, O) for r in res.results], axis=0
    )
    return (final, chunks), res


def kernel(x0, x1, x2, W, b, W_out, b_out):
    (final, chunks), _ = run(x0, x1, x2, W, b, W_out, b_out, trace=False)
    return final, chunks
